# revision 1
# baseline (speedup 1.0000x reference)
"""GCNConv (N=10000, E=640000, D=128) on 8 Trainium2 NeuronCores.

Math: out = D^{-1/2} (A + I) D^{-1/2} x W + bias, with deg computed over
edge_index[0] (+1 self-loop).  Rewritten as

    g   = diag(deg^-1/2) x                (pre-scale rows of x)
    agg[c] = sum_{e: col_e = c} g[row_e]  (incl. self loop)
    out[c] = deg^-1/2[c] * agg[c] @ W + bias

Device mapping (destination-sharded, 8 cores):
  - host: CSR by destination node, nodes sorted by in-degree, tiled into
    80 tiles of 128 nodes, each tile padded to its slot's max degree K_s.
    Tiles round-robined over cores by descending K so every core runs the
    SAME program (uniform K schedule) on different data.
  - device: build g table in DRAM (stream x, per-row scale), then per tile
    one dma_gather (128*K_s rows) + strided tensor_reduce over K + scale +
    transpose + matmul(W) + bias, write 128 output rows.
  - host: un-permute rows of the per-core outputs into the final array.
"""

import numpy as np

import concourse.bacc as bacc
import concourse.bass as bass
import concourse.mybir as mybir
import concourse.tile as tile
from concourse import bass_utils
from concourse.masks import make_identity

N_NODES = 10000
N_EDGES = 640000
D = 128
P = 128
NCORES = 8
NT = 79                      # ceil(N_NODES / P) real node tiles
NPAD = NT * P                # 10112
NTILES_TOTAL = 80            # padded to NCORES * SLOTS
SLOTS = NTILES_TOTAL // NCORES   # 10 tile-slots per core
XROWS = 10240                # g-table rows (80*128); rows >= N_NODES are zero
XTILE_ROWS = 1024            # rows per phase-B DMA (8 per partition)
NXT = XROWS // XTILE_ROWS    # 10 phase-B tiles
DUMMY = NPAD                 # gather index for padding slots (zero row)

f32 = mybir.dt.float32
i16 = mybir.dt.int16


def _build_schedule(edge_index: np.ndarray):
    """Host-side integer preprocessing: degrees, destination-CSR, degree-sorted
    tiling, uniform per-slot K schedule, per-core gather plans."""
    row = edge_index[0].astype(np.int64)
    col = edge_index[1].astype(np.int64)

    deg = np.bincount(row, minlength=N_NODES).astype(np.float32) + 1.0
    indeg = np.bincount(col, minlength=N_NODES).astype(np.int64) + 1

    indeg_pad = np.zeros(NPAD, np.int64)
    indeg_pad[:N_NODES] = indeg
    perm = np.argsort(indeg_pad, kind="stable")          # ascending in-degree

    # K per tile = max in-degree within the tile (sorted -> last element)
    Ks = indeg_pad[perm.reshape(NT, P)[:, -1]]
    Ks = np.maximum(Ks, 1)
    order = np.argsort(-Ks, kind="stable")               # tiles by K desc
    # slot s on every core uses K_slot[s] = max K among its rank group
    # (ranks are sorted by K desc, so that's the first real rank in the group)
    K_slot = [
        int(Ks[order[NCORES * s]]) if NCORES * s < NT else 1
        for s in range(SLOTS)
    ]
    # tile id for (core j, slot s); rank >= NT -> ghost tile (id = -1)
    tile_of = np.full((NCORES, SLOTS), -1, np.int64)
    for s in range(SLOTS):
        for j in range(NCORES):
            r = s * NCORES + j
            if r < NT:
                tile_of[j, s] = order[r]

    # destination-CSR: per-node source list (edges then self-loop)
    es = np.argsort(col, kind="stable")
    col_s = col[es]
    row_s = row[es]
    starts = np.searchsorted(col_s, np.arange(N_NODES), side="left")
    r_in = np.arange(N_EDGES) - starts[col_s]
    Kmax = int(indeg.max())
    M = np.full((NPAD, Kmax), DUMMY, np.int16)
    M[col_s, r_in] = row_s.astype(np.int16)
    M[np.arange(N_NODES), indeg - 1] = np.arange(N_NODES, dtype=np.int16)

    # per-core wrapped gather plans + sorted deg table + output node map
    idx_maps = []
    deg_sorted_maps = []
    node_maps = []          # [SLOTS*P] original node id per output row (-1 ghost)
    deg_pad = np.ones(NPAD, np.float32)
    deg_pad[:N_NODES] = deg
    for j in range(NCORES):
        flats = []
        degs = np.ones((P, SLOTS), np.float32)
        nmap = np.full(SLOTS * P, -1, np.int64)
        # dummy rows spread over the 128 zero rows 10112+p so padding
        # gathers don't all hammer one HBM row
        dummy_row = (DUMMY + np.arange(P, dtype=np.int16))[None, :]  # [1, P]
        for s in range(SLOTS):
            K = K_slot[s]
            t = tile_of[j, s]
            if t < 0:
                flats.append(np.broadcast_to(dummy_row, (K, P)).astype(
                    np.int16).ravel())
                continue
            nodes = perm[t * P : (t + 1) * P]
            plan = M[nodes, :K].T.copy()                 # [K, P]
            pad = plan == DUMMY
            plan[pad] = np.broadcast_to(dummy_row, plan.shape)[pad]
            flats.append(plan.ravel())                   # flat[k*P + p]
            degs[:, s] = deg_pad[nodes]
            real = nodes < N_NODES
            nmap[s * P : (s + 1) * P][real] = nodes[real]
        flat = np.concatenate(flats)                     # [P * sum(K_slot)]
        wrapped = flat.reshape(-1, 16).T                 # [16, L/16]
        idx_maps.append(np.tile(wrapped, (8, 1)).copy()) # replicate to 128 p
        deg_sorted_maps.append(degs)
        node_maps.append(nmap)

    # phase-B scale table: deg_cons[p, i*8+j] = deg of row i*1024 + 8p + j
    rows = (
        np.arange(NXT)[:, None, None] * XTILE_ROWS
        + 8 * np.arange(P)[None, :, None]
        + np.arange(8)[None, None, :]
    )  # [NXT, P, 8]
    deg_rows = np.ones(XROWS, np.float32)
    deg_rows[:N_NODES] = deg
    deg_cons = deg_rows[rows].transpose(1, 0, 2).reshape(P, NXT * 8).copy()

    return K_slot, idx_maps, deg_sorted_maps, node_maps, deg_cons


def _build_program(K_slot, loop_n=1, use_barrier=False, gdt=None,
                   gbufs=2, partial=False, sp=True):
    """Build the (uniform) 8-core SPMD Bass program for a given K schedule.
    loop_n > 1 repeats the whole computation on-device (benchmarking only).
    gdt: gather/g-table dtype (default f32). partial: per-chunk reduces."""
    if gdt is None:
        gdt = f32
    L16 = 8 * sum(K_slot)    # idx buffer free dim (int16 cols)

    nc = bacc.Bacc("TRN2", target_bir_lowering=False, debug=False,
                   num_devices=NCORES, num_swdge_queues=4)
    x_d = nc.dram_tensor("x_pad", [XROWS, D], f32, kind="ExternalInput")
    degc_d = nc.dram_tensor("deg_cons", [P, NXT * 8], f32, kind="ExternalInput")
    degs_d = nc.dram_tensor("deg_sorted", [P, SLOTS], f32, kind="ExternalInput")
    idx_d = nc.dram_tensor("idx", [P, L16], i16, kind="ExternalInput")
    w_d = nc.dram_tensor("W", [D, D], f32, kind="ExternalInput")
    b_d = nc.dram_tensor("bias", [1, D], f32, kind="ExternalInput")
    out_d = nc.dram_tensor("out", [SLOTS * P, D], f32, kind="ExternalOutput")
    g_d = nc.dram_tensor("g_table", [XROWS, D], gdt, kind="Internal")

    with tile.TileContext(nc) as tc:
        with (
            tc.tile_pool(name="const", bufs=1) as cpool,
            tc.tile_pool(name="xio", bufs=3) as xpool,
            tc.tile_pool(name="gth", bufs=gbufs) as gpool,
            tc.tile_pool(name="red", bufs=2) as rpool,
            tc.tile_pool(name="ps", bufs=2, space="PSUM") as ppool,
        ):

          def _emit_consts():
            # ---- phase A: constants ----
            idx_t = cpool.tile([P, L16], i16)
            nc.sync.dma_start(out=idx_t[:], in_=idx_d.ap())
            w_t = cpool.tile([D, D], f32)
            nc.sync.dma_start(out=w_t[:], in_=w_d.ap())
            bias_t = cpool.tile([P, D], f32)
            nc.sync.dma_start(out=bias_t[:], in_=b_d.ap()[0].partition_broadcast(P))
            ident = cpool.tile([P, P], f32)
            make_identity(nc, ident[:])

            degc_t = cpool.tile([P, NXT * 8], f32)
            nc.sync.dma_start(out=degc_t[:], in_=degc_d.ap())
            disc_t = cpool.tile([P, NXT * 8], f32)
            nc.vector.reciprocal(disc_t[:], degc_t[:])
            nc.scalar.sqrt(disc_t[:], disc_t[:])

            degs_t = cpool.tile([P, SLOTS], f32)
            nc.sync.dma_start(out=degs_t[:], in_=degs_d.ap())
            diss_t = cpool.tile([P, SLOTS], f32)
            nc.vector.reciprocal(diss_t[:], degs_t[:])
            nc.scalar.sqrt(diss_t[:], diss_t[:])

            return idx_t, w_t, bias_t, ident, disc_t, diss_t

          def _emit_body(idx_t, w_t, bias_t, ident, disc_t, diss_t):
            # ---- phase B: g = diag(deg^-1/2) x, streamed to DRAM ----
            # contiguous layout: partition p of tile i holds rows
            # i*1024 + 8p + j (j = 0..7), 4 KiB per partition per DMA
            x_v = x_d.ap().rearrange("(i p j) d -> i p (j d)", p=P, j=8)
            g_v = g_d.ap().rearrange("(i p j) d -> i p (j d)", p=P, j=8)
            for i in range(NXT):
                xt = xpool.tile([P, 8 * D], f32, tag="xt")
                nc.sync.dma_start(out=xt[:], in_=x_v[i])
                if gdt is f32:
                    ot = xt
                else:
                    ot = xpool.tile([P, 8 * D], gdt, tag="xt16")
                for jj in range(8):
                    nc.vector.tensor_scalar_mul(
                        ot[:, jj * D : (jj + 1) * D],
                        xt[:, jj * D : (jj + 1) * D],
                        disc_t[:, i * 8 + jj : i * 8 + jj + 1],
                    )
                nc.sync.dma_start(out=g_v[i], in_=ot[:])

            # all g writes must land before any gather reads (Tile tracks the
            # DRAM RAW deps; the explicit all-engine barrier is optional)
            if use_barrier:
                tc.strict_bb_all_engine_barrier()

            # ---- phase D: per node-tile gather + reduce + project ----
            # SWDGE descriptor ring holds ~1024 descs -> chunk each gather
            # to <= 1024 indices (8 k-slots of 128)
            CHUNK = 8
            off = 0
            qi = 0
            for s in range(SLOTS):
                K = K_slot[s]
                gt = gpool.tile([P, K * D], gdt, tag="gt")
                gt3 = gt[:].rearrange("p (k d) -> p k d", k=K)
                parts = []
                for k0 in range(0, K, CHUNK):
                    kn = min(CHUNK, K - k0)
                    nc.gpsimd.dma_gather(
                        out_ap=gt3[:, k0 : k0 + kn, :],
                        in_ap=g_d.ap(),
                        idxs_ap=idx_t[:, off + 8 * k0 : off + 8 * (k0 + kn)],
                        num_idxs=P * kn,
                        num_idxs_reg=P * kn,
                        elem_size=D,
                        queue_num=qi % 4,
                        single_packet=sp,
                    )
                    qi += 1
                    if partial:
                        pr = rpool.tile([P, D], f32, tag="pr")
                        nc.vector.tensor_reduce(
                            out=pr[:],
                            in_=gt3[:, k0 : k0 + kn, :].rearrange(
                                "p k d -> p d k"),
                            axis=mybir.AxisListType.X,
                            op=mybir.AluOpType.add,
                        )
                        parts.append(pr)
                off += 8 * K
                red = rpool.tile([P, D], f32, tag="red")
                if partial:
                    nc.vector.tensor_add(out=red[:], in0=parts[0][:],
                                         in1=parts[1][:])
                    for pr in parts[2:]:
                        nc.vector.tensor_add(out=red[:], in0=red[:], in1=pr[:])
                else:
                    nc.vector.tensor_reduce(
                        out=red[:],
                        in_=gt[:].rearrange("p (k d) -> p d k", k=K),
                        axis=mybir.AxisListType.X,
                        op=mybir.AluOpType.add,
                    )
                nc.vector.tensor_scalar_mul(red[:], red[:], diss_t[:, s : s + 1])
                redT_p = ppool.tile([P, P], f32, tag="tp")
                nc.tensor.transpose(out=redT_p[:], in_=red[:], identity=ident[:])
                redT = rpool.tile([P, P], f32, tag="redT")
                nc.vector.tensor_copy(out=redT[:], in_=redT_p[:])
                o_p = ppool.tile([P, D], f32, tag="op")
                nc.tensor.matmul(o_p[:], lhsT=redT[:], rhs=w_t[:],
                                 start=True, stop=True)
                o_t = rpool.tile([P, D], f32, tag="ot")
                nc.vector.tensor_add(out=o_t[:], in0=o_p[:], in1=bias_t[:])
                nc.sync.dma_start(
                    out=out_d.ap()[s * P : (s + 1) * P, :], in_=o_t[:]
                )

          consts = _emit_consts()
          if loop_n == 1:
              _emit_body(*consts)
          else:
              with tc.For_i(0, loop_n, 1):
                  _emit_body(*consts)

    nc.compile()
    return nc


def kernel(x, edge_index, W, bias):
    x = np.asarray(x, dtype=np.float32)
    edge_index = np.asarray(edge_index)
    W = np.asarray(W, dtype=np.float32)
    bias = np.asarray(bias, dtype=np.float32)
    assert x.shape == (N_NODES, D) and edge_index.shape == (2, N_EDGES)

    K_slot, idx_maps, deg_sorted_maps, node_maps, deg_cons = _build_schedule(
        edge_index
    )
    nc = _build_program(K_slot)

    x_pad = np.zeros((XROWS, D), np.float32)
    x_pad[:N_NODES] = x
    bias2 = bias.reshape(1, D).astype(np.float32)

    in_maps = []
    for j in range(NCORES):
        in_maps.append(
            {
                "x_pad": x_pad,
                "deg_cons": deg_cons,
                "deg_sorted": deg_sorted_maps[j],
                "idx": np.ascontiguousarray(idx_maps[j]),
                "W": W,
                "bias": bias2,
            }
        )

    res = bass_utils.run_bass_kernel_spmd(nc, in_maps, core_ids=list(range(NCORES)))

    out = np.zeros((N_NODES, D), np.float32)
    for j in range(NCORES):
        oj = res.results[j]["out"]
        nmap = node_maps[j]
        real = nmap >= 0
        out[nmap[real]] = oj[real]
    return out



# revision 3
# speedup vs baseline: 627.3912x; 627.3912x over previous
"""GCNConv (N=10000, E=640000, D=128) on 8 Trainium2 NeuronCores.

Math: out = diag(dis) (A + I) diag(dis) x W + bias, dis = deg^-1/2 with deg
over edge_index[0] (+1 self-loop).  Since the edge weight factorizes as
dis[row]*dis[col], fold dis[row] into a host-prescaled table
g = diag(dis) x  and dis[col] into a post-scale.  The aggregation then
becomes a sum of DENSE block matmuls against an integer-count adjacency:

    aggT[d, c] = sum_j  g_j[s, d]^T  @  A_j[s, c]      (PSUM accumulate)
    outT = (W^T @ aggT) * dis[col] + bias[d_out]        (on-device tail)

Device mapping (destination-sharded, 8 cores, SPMD):
  - 80 node tiles of 128; core j owns 10 consecutive dest tiles (1280 cols).
  - A blocks are {0,1,2,..} edge counts, EXACT in fp8e4 -> rhs stream is
    12.6 MB/core of sequential HBM reads (no gather, no SWDGE descriptors).
  - g is f16 [128, 80*128] (partition-major, host-prepped); lhsT = g_j.
  - 80 x 3 accumulating matmuls into three PSUM regions [d, 512|512|256].
  - tail: PSUM -> f16, 3 W-matmuls (f16), scale by dis[col] (broadcast
    tile), + bias (per-partition), write outT [128, 1280]; host transposes.
"""

import numpy as np

import concourse.bacc as bacc
import concourse.bass as bass
import concourse.mybir as mybir
import concourse.tile as tile
from concourse import bass_utils

N_NODES = 10000
N_EDGES = 640000
D = 128
P = 128
NCORES = 8
NT = 80                  # node tiles (src and dest)
NPAD = NT * P            # 10240
TPC = NT // NCORES       # 10 dest tiles per core
CPC = TPC * P            # 1280 dest columns per core
ACH = 8                  # src tiles per A-stream chunk
NCH = NT // ACH          # 10 chunks
CG = (512, 512, 256)     # dest column groups per matmul (PSUM bank limit)

f32 = mybir.dt.float32
f16 = mybir.dt.float16
f8 = mybir.dt.float8e4


def _build_inputs(x, edge_index, W, bias):
    """Host-side prep: prescaled g table (f16, partition-major), per-core
    fp8 adjacency-count blocks, per-core dest scales."""
    row = edge_index[0].astype(np.int64)
    col = edge_index[1].astype(np.int64)

    deg = np.bincount(row, minlength=N_NODES).astype(np.float64) + 1.0
    dis = (deg ** -0.5).astype(np.float32)
    dis_pad = np.zeros(NPAD, np.float32)
    dis_pad[:N_NODES] = dis

    g_pad = np.zeros((NPAD, D), np.float32)
    g_pad[:N_NODES] = x * dis[:, None]
    g_sb = np.ascontiguousarray(
        g_pad.reshape(NT, P, D).transpose(1, 0, 2).reshape(P, NT * D)
    ).astype(np.float16)

    f8np = mybir.dt.np(f8)
    W16 = W.astype(np.float16)
    bias_p = np.ascontiguousarray(bias.reshape(D, 1)).astype(np.float32)

    in_maps = []
    for j in range(NCORES):
        lo, hi = j * CPC, (j + 1) * CPC
        m = (col >= lo) & (col < hi)
        r = row[m]
        c = col[m] - lo
        sl = np.arange(lo, min(hi, N_NODES), dtype=np.int64)
        rr = np.concatenate([r, sl])
        cc = np.concatenate([c, sl - lo])
        cnt = np.bincount(rr * CPC + cc, minlength=NPAD * CPC)
        mx = cnt.max()
        assert mx <= 8, f"edge multiplicity {mx} not exact in fp8e4"
        A = np.ascontiguousarray(
            cnt.reshape(NT, P, CPC).transpose(1, 0, 2).reshape(P, NT * CPC)
        ).astype(np.float32).astype(f8np)
        in_maps.append(
            {
                "g_sb": g_sb,
                "A": A,
                "W16": W16,
                "diss": dis_pad[lo:hi].reshape(1, CPC).copy(),
                "bias_p": bias_p,
            }
        )
    return in_maps


def _build_program(loop_n=1):
    nc = bacc.Bacc("TRN2", target_bir_lowering=False, debug=False,
                   num_devices=NCORES)
    g_d = nc.dram_tensor("g_sb", [P, NT * D], f16, kind="ExternalInput")
    a_d = nc.dram_tensor("A", [P, NT * CPC], f8, kind="ExternalInput")
    w_d = nc.dram_tensor("W16", [D, D], f16, kind="ExternalInput")
    diss_d = nc.dram_tensor("diss", [1, CPC], f32, kind="ExternalInput")
    bias_d = nc.dram_tensor("bias_p", [D, 1], f32, kind="ExternalInput")
    out_d = nc.dram_tensor("outT", [P, CPC], f32, kind="ExternalOutput")

    with tile.TileContext(nc) as tc:
        with (
            tc.tile_pool(name="const", bufs=1) as cpool,
            tc.tile_pool(name="astr", bufs=3) as apool,
            tc.tile_pool(name="tail", bufs=2) as spool,
            tc.tile_pool(name="pacc", bufs=1, space="PSUM") as pgpool,
            tc.tile_pool(name="pout", bufs=1, space="PSUM") as popool,
        ):

            def _consts():
                g_t = cpool.tile([P, NT * D], f16)
                nc.sync.dma_start(out=g_t[:], in_=g_d.ap())
                w_t = cpool.tile([D, D], f16)
                nc.sync.dma_start(out=w_t[:], in_=w_d.ap())
                diss_b = cpool.tile([P, CPC], f32)
                nc.sync.dma_start(
                    out=diss_b[:], in_=diss_d.ap()[0].partition_broadcast(P)
                )
                bias_t = cpool.tile([P, 1], f32)
                nc.sync.dma_start(out=bias_t[:], in_=bias_d.ap())
                return g_t, w_t, diss_b, bias_t

            def _body(g_t, w_t, diss_b, bias_t):
                pg = [pgpool.tile([P, n], f32, tag=f"pg{k}", name=f"pg{k}")
                      for k, n in enumerate(CG)]
                for jc in range(NCH):
                    a_t = apool.tile([P, ACH * CPC], f8, tag="a")
                    nc.sync.dma_start(
                        out=a_t[:],
                        in_=a_d.ap()[:, jc * ACH * CPC:(jc + 1) * ACH * CPC],
                    )
                    for jl in range(ACH):
                        j = jc * ACH + jl
                        lhs = g_t[:, j * D:(j + 1) * D]
                        base = jl * CPC
                        off = 0
                        for k, n in enumerate(CG):
                            nc.tensor.matmul(
                                pg[k][:],
                                lhsT=lhs,
                                rhs=a_t[:, base + off:base + off + n],
                                start=(j == 0),
                                stop=(j == NT - 1),
                            )
                            off += n

                aggT = spool.tile([P, CPC], f16, tag="aggT")
                off = 0
                for k, n in enumerate(CG):
                    nc.vector.tensor_copy(out=aggT[:, off:off + n],
                                          in_=pg[k][:])
                    off += n
                po = [popool.tile([P, n], f32, tag=f"po{k}", name=f"po{k}")
                      for k, n in enumerate(CG)]
                off = 0
                for k, n in enumerate(CG):
                    nc.tensor.matmul(po[k][:], lhsT=w_t[:],
                                     rhs=aggT[:, off:off + n],
                                     start=True, stop=True)
                    off += n
                o_t = spool.tile([P, CPC], f32, tag="o")
                off = 0
                for k, n in enumerate(CG):
                    nc.vector.tensor_mul(out=o_t[:, off:off + n],
                                         in0=po[k][:],
                                         in1=diss_b[:, off:off + n])
                    off += n
                nc.vector.tensor_scalar_add(o_t[:], o_t[:], bias_t[:, 0:1])
                nc.sync.dma_start(out=out_d.ap(), in_=o_t[:])

            consts = _consts()
            if loop_n == 1:
                _body(*consts)
            else:
                with tc.For_i(0, loop_n, 1):
                    _body(*consts)

    nc.compile()
    return nc


def kernel(x, edge_index, W, bias):
    x = np.asarray(x, dtype=np.float32)
    edge_index = np.asarray(edge_index)
    W = np.asarray(W, dtype=np.float32)
    bias = np.asarray(bias, dtype=np.float32)
    assert x.shape == (N_NODES, D) and edge_index.shape == (2, N_EDGES)

    in_maps = _build_inputs(x, edge_index, W, bias)
    nc = _build_program()
    res = bass_utils.run_bass_kernel_spmd(nc, in_maps,
                                          core_ids=list(range(NCORES)))

    out = np.empty((NCORES * CPC, D), np.float32)
    for j in range(NCORES):
        out[j * CPC:(j + 1) * CPC] = res.results[j]["outT"].T
    return out[:N_NODES]
